# revision 1
# baseline (speedup 1.0000x reference)
"""Baichuan attention prefill (q_len=2048, H=5120, 40 heads) on 8 Trainium2
NeuronCores, tensor-parallel over heads (5 heads/core), all-reduce on host.

v7: restructured from v6 for HW-side LDWEIGHTS amortization, halved DMA, and
a shorter phase-2 cross-engine chain.

  Phase 1: qkvT [1920, 2048] = W_c @ X^T over TWO 1024-token x-pairs (x pair
           resident, full 5120 contraction, two 512-wide PSUM tiles per
           m-tile sharing each weight load) -> halves weight DMA (78->39 MB)
           and doubles moving rows per stationary.  v-tiles additionally
           PE-transposed to token-major v_sb during phase 1 (PE/PSUM slack).
  Phase 2: per-head attention with a multiplicative mask: host precomputes
           em = exp(mask - 8) (f16); ACT computes eu = exp(scores - 4)
           straight from PSUM (no DVE f32 add), DVE forms
           et = eu * em at 2x f16 speed.  et = e^{s+m-12} exactly as v6.
           Denominator via ones-matrix [128,128] stationary -> sums arrive
           broadcast across all 128 partitions (no gpsimd chain);
           normalization = DVE reciprocal + multiply into f16 attnT.
           Masks stream per (j, it) slice, double-buffered two groups ahead.
  Phase 3: row-parallel o_proj with wo STATIONARY: one weight load per 4
           512-wide matmuls; output yT [5120, 2048] f16 (halves store DMA);
           partials summed + transposed on host.
"""

import math
import numpy as np

import concourse.bass as bass
import concourse.mybir as mybir
import concourse.tile as tile
from concourse import bacc
from concourse.bass_utils import run_bass_kernel_spmd
from concourse.masks import make_identity

H = 5120
NH = 40
HD = 128
T = 2048
NCORES = 8
HPC = NH // NCORES          # 5 heads per core
DPC = HPC * HD              # 640 channels per core
KC = H // 128               # 40 contraction chunks
NPAIR = 2                   # x token pairs (1024 each)
PB = T // NPAIR             # 1024 tokens per pair
JC = T // 128               # 16 key chunks
ITN = T // 512              # 4 query tiles

F32 = mybir.dt.float32
F16 = mybir.dt.float16
EXP = mybir.ActivationFunctionType.Exp

EXP_SHIFT_S = -4.0          # eu = exp(s - 4);  em = exp(m - 8) on host
                            # product = e^{s+m-12}: same shifted weights as v6

# m-tile m holds (q|k|v of head m//3): already head-interleaved in layout
M_ORDER = list(range(3 * HPC))


def _phase1(nc, tc, xP, wP, qkv_sb, v_sb, ident, rep=0):
    """qkvT channel-major into resident f16 SBUF tiles; two 512-token PSUM
    tiles per m-tile share one weight load; v transposed to token-major."""
    with tc.tile_pool(name=f"p1x_{rep}", bufs=1) as xp, \
         tc.tile_pool(name=f"p1w_{rep}", bufs=2) as wp, \
         tc.tile_pool(name=f"p1ps_{rep}", bufs=4, space="PSUM") as pp, \
         tc.tile_pool(name=f"p1vt_{rep}", bufs=2, space="PSUM") as vtp:
        for np_ in range(NPAIR):
            # two half-pair tiles: half A is released by the last psa run well
            # before half B's psb runs finish, so the next pair's A-half DMA
            # overlaps this pair's tail instead of stalling at the boundary
            xba = xp.tile([128, KC, 512], F16, tag="xba", name=f"xba_{rep}_{np_}")
            xbb = xp.tile([128, KC, 512], F16, tag="xbb", name=f"xbb_{rep}_{np_}")
            for half, xh in ((0, xba), (1, xbb)):
                for q in range(4):  # quarter-granularity arrival
                    k0, k1 = q * (KC // 4), (q + 1) * (KC // 4)
                    nc.sync.dma_start(
                        out=xh[:, k0:k1, :],
                        in_=xP[np_, :, k0 * PB:k1 * PB]
                        .rearrange("p (k t) -> p k t", t=PB)
                        [:, :, half * 512:(half + 1) * 512])
            for m in M_ORDER:
                wm = wp.tile([128, KC, 128], F16, tag="w", name=f"w_{rep}_{np_}_{m}")
                nc.scalar.dma_start(
                    out=wm,
                    in_=wP[m].rearrange("p (k c) -> p k c", c=128))
                # two sequential same-bank accumulation runs (NOT interleaved:
                # per-MM psum-bank cycling trips the HAM throttle / K18 mode)
                psa = pp.tile([128, 512], F32, tag="qkps", name=f"qkpsa_{rep}_{np_}_{m}")
                psb = pp.tile([128, 512], F32, tag="qkps", name=f"qkpsb_{rep}_{np_}_{m}")
                for k in range(KC):
                    nc.tensor.matmul(psa, wm[:, k, :], xba[:, k, :],
                                     start=(k == 0), stop=(k == KC - 1))
                for k in range(KC):
                    nc.tensor.matmul(psb, wm[:, k, :], xbb[:, k, :],
                                     start=(k == 0), stop=(k == KC - 1))
                dst = qkv_sb[m]
                nc.scalar.copy(dst[:, np_ * PB:np_ * PB + 512], psa)
                nc.scalar.copy(dst[:, np_ * PB + 512:(np_ + 1) * PB], psb)
                if m % 3 == 2:      # v-tile: transpose to token-major v_sb
                    h = m // 3
                    for jj in range(PB // 128):
                        j = np_ * (PB // 128) + jj
                        vt_ps = vtp.tile([128, 128], F16, tag="vt",
                                         name=f"vtps_{rep}_{np_}_{m}_{jj}")
                        nc.tensor.transpose(
                            vt_ps, dst[:, j * 128:(j + 1) * 128], ident)
                        nc.scalar.copy(v_sb[j][:, h * 128:(h + 1) * 128], vt_ps)


def _phase2(nc, tc, qkv_sb, v_sb, emT, attnT, ones_mat, bias_sh, rep=0):
    """Per-head fused attention from resident qkv into persistent attnT."""
    with tc.tile_pool(name=f"p2m_{rep}", bufs=2) as mp, \
         tc.tile_pool(name=f"p2eu_{rep}", bufs=6) as eup, \
         tc.tile_pool(name=f"p2e_{rep}", bufs=8) as ep, \
         tc.tile_pool(name=f"p2rec_{rep}", bufs=2) as rcp, \
         tc.tile_pool(name=f"p2sc_{rep}", bufs=3, space="PSUM") as scp, \
         tc.tile_pool(name=f"p2acc_{rep}", bufs=3, space="PSUM") as accp:
        for h in range(HPC):
            qT = qkv_sb[3 * h]
            kT = qkv_sb[3 * h + 1]
            for g in range(ITN // 2):
                its = (2 * g, 2 * g + 1)
                # mask em tiles for this (h, g): 16 j x 2 its, each [128, 512]
                mts = {}
                for j in range(JC):
                    for u, it in enumerate(its):
                        mt = mp.tile([128, 512], F16, tag=f"em{j}_{u}",
                                     name=f"em_{rep}_{h}_{g}_{j}_{u}")
                        nc.sync.dma_start(
                            out=mt,
                            in_=emT[h, j * 128:(j + 1) * 128,
                                    it * 512:(it + 1) * 512])
                        mts[(j, it)] = mt
                aps, sps = {}, {}
                for it in its:
                    aps[it] = accp.tile([128, 512], F32, tag="attnps",
                                        name=f"attnps_{rep}_{h}_{it}")
                    sps[it] = accp.tile([128, 512], F32, tag="sumps",
                                        name=f"sumps_{rep}_{h}_{it}", bufs=2)
                for j in range(JC):
                    ets = {}
                    for it in its:
                        sc = scp.tile([128, 512], F32, tag="scps",
                                      name=f"scps_{rep}_{h}_{it}_{j}")
                        nc.tensor.matmul(sc, kT[:, j * 128:(j + 1) * 128],
                                         qT[:, it * 512:(it + 1) * 512],
                                         start=True, stop=True)
                        eu = eup.tile([128, 512], F16, tag="eu",
                                      name=f"eu_{rep}_{h}_{it}_{j}")
                        nc.scalar.activation(eu, sc, EXP, bias=bias_sh)
                        et = ep.tile([128, 512], F16, tag="e",
                                     name=f"e_{rep}_{h}_{it}_{j}")
                        nc.vector.tensor_mul(et, eu, mts[(j, it)])
                        ets[it] = et
                    for it in its:
                        nc.tensor.matmul(aps[it], v_sb[j][:, h * 128:(h + 1) * 128],
                                         ets[it], start=(j == 0), stop=(j == JC - 1))
                    for it in its:
                        nc.tensor.matmul(sps[it], ones_mat, ets[it],
                                         start=(j == 0), stop=(j == JC - 1))
                for it in its:
                    rec = rcp.tile([128, 512], F32, tag="rec",
                                   name=f"rec_{rep}_{h}_{it}")
                    nc.vector.reciprocal(rec, sps[it])
                    nc.vector.tensor_mul(attnT[h][:, it * 512:(it + 1) * 512],
                                         aps[it], rec)


def _phase3(nc, tc, attnT, woP, yT, wp, op, rep=0):
    """Row-parallel o_proj, wo stationary: one weight load per 4 matmuls.
    yT[o, i] = sum_dh wo[dh, o] * attnT[dh, i], stored f16 transposed."""
    OSUB = H // 128         # 40 output row tiles
    with tc.tile_pool(name=f"p3ps_{rep}", bufs=8, space="PSUM") as pp:
        for o in range(OSUB):
            wo = wp.tile([128, HPC, 128], F16, tag="wo", name=f"wo_{rep}_{o}")
            nc.gpsimd.dma_start(
                out=wo,
                in_=woP[o].rearrange("p (c m) -> p c m", m=128))
            ps = [pp.tile([128, 512], F32, tag="yps", name=f"yps_{rep}_{o}_{i}")
                  for i in range(4)]
            yo = op.tile([128, 4, 512], F16, tag="yo", name=f"yo_{rep}_{o}")
            # i-outer: each PSUM bank's 5-matmul accumulation runs contiguously
            for i in range(4):
                for c in range(HPC):
                    nc.tensor.matmul(ps[i], wo[:, c, :],
                                     attnT[c][:, i * 512:(i + 1) * 512],
                                     start=(c == 0), stop=(c == HPC - 1))
                if i % 2 == 0:
                    nc.scalar.copy(yo[:, i, :], ps[i])
                else:
                    nc.vector.tensor_copy(yo[:, i, :], ps[i])
            nc.gpsimd.dma_start(
                out=yT[o * 128:(o + 1) * 128, :],
                in_=yo.rearrange("p i t -> p (i t)"))


def build(repeat=1, phases=(1, 2, 3)):
    nc = bacc.Bacc("TRN2", target_bir_lowering=False, debug=False, num_devices=NCORES)
    # per-tile contiguous packed layouts (host pre-shuffles): every DMA reads
    # >=1.25KB contiguous per partition
    xP = nc.dram_tensor("xP", [NPAIR, 128, KC * PB], F16, kind="ExternalInput").ap()
    wP = nc.dram_tensor("wP", [3 * HPC, 128, KC * 128], F16, kind="ExternalInput").ap()
    woP = nc.dram_tensor("woP", [H // 128, 128, HPC * 128], F16, kind="ExternalInput").ap()
    emT = nc.dram_tensor("emT", [HPC, T, T], F16, kind="ExternalInput").ap()
    yT = nc.dram_tensor("yT", [H, T], F16, kind="ExternalOutput").ap()

    with tile.TileContext(nc) as tc:
        with tc.tile_pool(name="qkvp", bufs=1) as qp, \
             tc.tile_pool(name="vsbp", bufs=1) as vp, \
             tc.tile_pool(name="attnTp", bufs=1) as ap, \
             tc.tile_pool(name="constp", bufs=1) as cp:
            ones_f = cp.tile([128, 128], F32, name="ones_f")
            nc.vector.memset(ones_f, 1.0)
            ones_mat = cp.tile([128, 128], F16, name="ones_mat")
            nc.vector.tensor_copy(ones_mat, ones_f)
            ident_f = cp.tile([128, 128], F32, name="ident_f")
            make_identity(nc, ident_f)
            ident = cp.tile([128, 128], F16, name="ident")
            nc.vector.tensor_copy(ident, ident_f)
            bias_sh = cp.tile([128, 1], F32, name="bias_sh")
            nc.vector.memset(bias_sh, EXP_SHIFT_S)
            qkv_sb = [qp.tile([128, T], F16, name=f"qkv_{m}") for m in range(3 * HPC)]
            v_sb = [vp.tile([128, DPC], F16, name=f"vsb_{j}") for j in range(JC)]
            attnT = [ap.tile([128, T], F16, name=f"attnT_{c}") for c in range(HPC)]
            for rep in range(repeat):
                if 1 in phases:
                    _phase1(nc, tc, xP, wP, qkv_sb, v_sb, ident, rep)
                with tc.tile_pool(name=f"p3w_{rep}", bufs=2) as wp3, \
                     tc.tile_pool(name=f"p3o_{rep}", bufs=2) as op3:
                    if 2 in phases:
                        _phase2(nc, tc, qkv_sb, v_sb, emT, attnT, ones_mat,
                                bias_sh, rep)
                    if 3 in phases:
                        _phase3(nc, tc, attnT, woP, yT, wp3, op3, rep)
    nc.compile()
    return nc


_nc = None


def _get_nc():
    global _nc
    if _nc is None:
        _nc = build()
    return _nc


def make_in_maps(hidden_states, attention_mask, W_pack, o_proj_w):
    hs = np.ascontiguousarray(np.asarray(hidden_states, dtype=np.float32).reshape(T, H))
    mask = np.asarray(attention_mask, dtype=np.float32)
    wp = np.asarray(W_pack, dtype=np.float32)
    wo = np.asarray(o_proj_w, dtype=np.float32)

    xT = hs.T.astype(np.float16)                          # [H, T]
    # packed x: xP[n, p, k*PB+t] = xT[k*128+p, n*PB+t]
    xP = np.ascontiguousarray(
        xT.reshape(KC, 128, NPAIR, PB).transpose(2, 1, 0, 3)
        .reshape(NPAIR, 128, KC * PB))
    scale = np.float32(1.0 / math.sqrt(HD))
    wq = wp[0:H].reshape(NH, HD, H)
    wk = wp[H:2 * H].reshape(NH, HD, H)
    wv = wp[2 * H:3 * H].reshape(NH, HD, H)

    in_maps = []
    for c in range(NCORES):
        h0, h1 = c * HPC, (c + 1) * HPC
        # m-tile i holds (q|k|v of head i//3) matching M_ORDER's qkv_sb index
        blocks = []
        for h in range(h0, h1):
            blocks.append(wq[h] * scale)
            blocks.append(wk[h])
            blocks.append(wv[h])
        w_c = np.concatenate(blocks, axis=0)              # [1920, H]
        wqkvT_c = w_c.T.astype(np.float16)                # [H, 1920]
        # packed w: wP[m, p, k*128+col] = wqkvT[k*128+p, m*128+col]
        wP_c = np.ascontiguousarray(
            wqkvT_c.reshape(KC, 128, 3 * HPC, 128).transpose(2, 1, 0, 3)
            .reshape(3 * HPC, 128, KC * 128))
        woT_c = wo[:, h0 * HD:h1 * HD].T.astype(np.float16)   # [640, H]
        # packed wo: woP[o, p, cblk*128+col] = woT[cblk*128+p, o*128+col]
        woP_c = np.ascontiguousarray(
            woT_c.reshape(HPC, 128, H // 128, 128).transpose(2, 1, 0, 3)
            .reshape(H // 128, 128, HPC * 128))
        emT_c = np.ascontiguousarray(
            np.exp(mask[h0:h1].transpose(0, 2, 1) - 8.0).astype(np.float16))
        in_maps.append({"xP": xP, "wP": wP_c, "woP": woP_c, "emT": emT_c})
    return in_maps


_runner = None


def _cached_runner(nc):
    """Jit the bass_exec shard_map once so repeat kernel() calls skip the
    walrus/NEFF recompile that a fresh run_bass_kernel_spmd would pay."""
    import jax
    from jax.experimental.shard_map import shard_map
    from jax.sharding import Mesh, PartitionSpec
    from concourse import bass2jax

    bass2jax.install_neuronx_cc_hook()
    partition_name = nc.partition_id_tensor.name if nc.partition_id_tensor else None
    in_names, out_names, out_avals, zero_outs = [], [], [], []
    for alloc in nc.m.functions[0].allocations:
        if not isinstance(alloc, mybir.MemoryLocationSet):
            continue
        name = alloc.memorylocations[0].name
        if alloc.kind == "ExternalInput":
            if name != partition_name:
                in_names.append(name)
        elif alloc.kind == "ExternalOutput":
            out_names.append(name)
            shape = tuple(alloc.tensor_shape)
            dtype = mybir.dt.np(alloc.dtype)
            out_avals.append(jax.core.ShapedArray(shape, dtype))
            zero_outs.append(np.zeros(shape, dtype))
    all_in = list(in_names) + list(out_names)
    if partition_name is not None:
        all_in.append(partition_name)

    def _body(*args):
        operands = list(args)
        if partition_name is not None:
            operands.append(bass2jax.partition_id_tensor())
        outs = bass2jax._bass_exec_p.bind(
            *operands, out_avals=tuple(out_avals), in_names=tuple(all_in),
            out_names=tuple(out_names), lowering_input_output_aliases=(),
            sim_require_finite=True, sim_require_nnan=True, nc=nc)
        return tuple(outs)

    mesh = Mesh(np.asarray(jax.devices()[:NCORES]), ("core",))
    n_args = len(in_names) + len(out_names)
    fn = jax.jit(shard_map(_body, mesh=mesh,
                           in_specs=(PartitionSpec("core"),) * n_args,
                           out_specs=(PartitionSpec("core"),) * len(out_names),
                           check_rep=False), keep_unused=True)

    def run(in_maps):
        args = [np.concatenate([np.asarray(m[n]) for m in in_maps], axis=0)
                for n in in_names]
        args += [np.zeros((NCORES * z.shape[0], *z.shape[1:]), z.dtype)
                 for z in zero_outs]
        outs = fn(*args)
        return [{name: np.asarray(outs[i]).reshape(NCORES, *out_avals[i].shape)[c]
                 for i, name in enumerate(out_names)} for c in range(NCORES)]

    return run


def kernel(input_pos=None, end=None, hidden_states=None, attention_mask=None,
           W_pack=None, o_proj_w=None, k_cache=None, v_cache=None):
    # input_pos == arange(T) and end == T per the problem spec, so the KV
    # cache write is a full overwrite and the zero-filled caches never
    # contribute to the output — both are intentionally unused here.
    global _runner
    in_maps = make_in_maps(hidden_states, attention_mask, W_pack, o_proj_w)
    nc = _get_nc()
    if _runner is None:
        results = run_bass_kernel_spmd(nc, in_maps, list(range(NCORES))).results
        _runner = _cached_runner(nc)
    else:
        results = _runner(in_maps)
    y = results[0]["yT"].astype(np.float32)
    for c in range(1, NCORES):
        y = y + results[c]["yT"]
    return np.ascontiguousarray(y.T).reshape(1, T, H)



# revision 4
# speedup vs baseline: 1.0692x; 1.0692x over previous
"""Baichuan attention prefill (q_len=2048, H=5120, 40 heads) on 8 Trainium2
NeuronCores, tensor-parallel over heads (5 heads/core), all-reduce on host.

v9: v7 + stationary-adjacency restructure.  HW showed phase 1 at ~2.2x the
TimelineSim model (1011us vs ~530) while phases 2/3 matched it; micro-
benchmarks (work/microbench.py) ruled out DMA bandwidth (60MB in 58us) and
DMA/PE contention (pedma 463us) — the excess tracks per-matmul LDWEIGHTS
issue cost, which consecutive same-stationary matmuls amortize.

  Phase 1: qkvT [1920, 2048] = W_c @ X^T over TWO 1024-token x-pairs.
           k-OUTER interleave: each wm[:,k,:] stationary feeds the psa and
           psb 512-token matmuls back-to-back (ldweights halved); x halves
           DMA'd on separate queues (SP + Pool); PSUM pool 6 bufs; psa/psb
           evacuation split across ACT and DVE.  v-tiles PE-transposed to
           token-major v_sb as in v7.
  Phase 2: per-head attention with multiplicative mask em = exp(mask - 8)
           (host, f16); ACT eu = exp(scores - 4) from PSUM; DVE et = eu*em.
           Denominator ones-matmuls hoisted to a SECOND pass per (h, g):
           16-long same-bank accumulation runs with the ones stationary
           constant across the run (et tiles kept alive, pool bufs 17x2).
           Normalization = DVE reciprocal + multiply into f16 attnT.
  Phase 3: row-parallel o_proj, c-OUTER: one wo[:,c,:] stationary feeds 4
           consecutive 512-wide matmuls (i-banks cycled); yT f16 transposed,
           partials summed + transposed on host.
"""

import math
import numpy as np

import concourse.bass as bass
import concourse.mybir as mybir
import concourse.tile as tile
from concourse import bacc
from concourse.bass_utils import run_bass_kernel_spmd
from concourse.masks import make_identity

H = 5120
NH = 40
HD = 128
T = 2048
NCORES = 8
HPC = NH // NCORES          # 5 heads per core
DPC = HPC * HD              # 640 channels per core
KC = H // 128               # 40 contraction chunks
NPAIR = 2                   # x token pairs (1024 each)
PB = T // NPAIR             # 1024 tokens per pair
JC = T // 128               # 16 key chunks
ITN = T // 512              # 4 query tiles

F32 = mybir.dt.float32
F16 = mybir.dt.float16
EXP = mybir.ActivationFunctionType.Exp

EXP_SHIFT_S = -4.0          # eu = exp(s - 4);  em = exp(m - 8) on host
                            # product = e^{s+m-12}: same shifted weights as v6

# m-tile m holds (q|k|v of head m//3): already head-interleaved in layout
M_ORDER = list(range(3 * HPC))


def _phase1(nc, tc, xP, wP, qkv_sb, v_sb, ident, rep=0):
    """qkvT channel-major into resident f16 SBUF tiles; two 512-token PSUM
    tiles per m-tile share one weight load; v transposed to token-major."""
    with tc.tile_pool(name=f"p1x_{rep}", bufs=1) as xp, \
         tc.tile_pool(name=f"p1w_{rep}", bufs=2) as wp, \
         tc.tile_pool(name=f"p1ps_{rep}", bufs=6, space="PSUM") as pp, \
         tc.tile_pool(name=f"p1vt_{rep}", bufs=2, space="PSUM") as vtp:
        for np_ in range(NPAIR):
            # two half-pair tiles: half A is released by the last psa run well
            # before half B's psb runs finish, so the next pair's A-half DMA
            # overlaps this pair's tail instead of stalling at the boundary
            xba = xp.tile([128, KC, 512], F16, tag="xba", name=f"xba_{rep}_{np_}")
            xbb = xp.tile([128, KC, 512], F16, tag="xbb", name=f"xbb_{rep}_{np_}")
            for half, xh in ((0, xba), (1, xbb)):
                for q in range(4):  # quarter-granularity arrival
                    k0, k1 = q * (KC // 4), (q + 1) * (KC // 4)
                    eng = (nc.sync, nc.gpsimd)[half]
                    eng.dma_start(
                        out=xh[:, k0:k1, :],
                        in_=xP[np_, :, k0 * PB:k1 * PB]
                        .rearrange("p (k t) -> p k t", t=PB)
                        [:, :, half * 512:(half + 1) * 512])
            for m in M_ORDER:
                wm = wp.tile([128, KC, 128], F16, tag="w", name=f"w_{rep}_{np_}_{m}")
                nc.scalar.dma_start(
                    out=wm,
                    in_=wP[m].rearrange("p (k c) -> p k c", c=128))
                # k-outer interleave: one stationary feeds both 512-token
                # halves back-to-back (ldweights amortized 2x)
                psa = pp.tile([128, 512], F32, tag="qkps", name=f"qkpsa_{rep}_{np_}_{m}")
                psb = pp.tile([128, 512], F32, tag="qkps", name=f"qkpsb_{rep}_{np_}_{m}")
                for k in range(KC):
                    nc.tensor.matmul(psa, wm[:, k, :], xba[:, k, :],
                                     start=(k == 0), stop=(k == KC - 1))
                    nc.tensor.matmul(psb, wm[:, k, :], xbb[:, k, :],
                                     start=(k == 0), stop=(k == KC - 1))
                dst = qkv_sb[m]
                nc.scalar.copy(dst[:, np_ * PB:np_ * PB + 512], psa)
                nc.vector.tensor_copy(dst[:, np_ * PB + 512:(np_ + 1) * PB], psb)
                if m % 3 == 2:      # v-tile: transpose to token-major v_sb
                    h = m // 3
                    for jj in range(PB // 128):
                        j = np_ * (PB // 128) + jj
                        vt_ps = vtp.tile([128, 128], F16, tag="vt",
                                         name=f"vtps_{rep}_{np_}_{m}_{jj}")
                        nc.tensor.transpose(
                            vt_ps, dst[:, j * 128:(j + 1) * 128], ident)
                        nc.scalar.copy(v_sb[j][:, h * 128:(h + 1) * 128], vt_ps)


def _phase2(nc, tc, qkv_sb, v_sb, emT, attnT, ones_mat, bias_sh, rep=0):
    """Per-head fused attention from resident qkv into persistent attnT."""
    with tc.tile_pool(name=f"p2m_{rep}", bufs=1) as mp, \
         tc.tile_pool(name=f"p2eu_{rep}", bufs=6) as eup, \
         tc.tile_pool(name=f"p2e_{rep}", bufs=17) as ep, \
         tc.tile_pool(name=f"p2rec_{rep}", bufs=2) as rcp, \
         tc.tile_pool(name=f"p2sc_{rep}", bufs=3, space="PSUM") as scp, \
         tc.tile_pool(name=f"p2acc_{rep}", bufs=3, space="PSUM") as accp:
        for h in range(HPC):
            qT = qkv_sb[3 * h]
            kT = qkv_sb[3 * h + 1]
            for g in range(ITN // 2):
                its = (2 * g, 2 * g + 1)
                # mask em tiles for this (h, g): 16 j x 2 its, each [128, 512]
                mts = {}
                for j in range(JC):
                    for u, it in enumerate(its):
                        mt = mp.tile([128, 512], F16, tag=f"em{j}_{u}",
                                     name=f"em_{rep}_{h}_{g}_{j}_{u}")
                        nc.sync.dma_start(
                            out=mt,
                            in_=emT[h, j * 128:(j + 1) * 128,
                                    it * 512:(it + 1) * 512])
                        mts[(j, it)] = mt
                aps, sps = {}, {}
                for it in its:
                    aps[it] = accp.tile([128, 512], F32, tag="attnps",
                                        name=f"attnps_{rep}_{h}_{it}")
                    sps[it] = accp.tile([128, 512], F32, tag="sumps",
                                        name=f"sumps_{rep}_{h}_{it}", bufs=2)
                ets = {}
                for j in range(JC):
                    for u, it in enumerate(its):
                        sc = scp.tile([128, 512], F32, tag="scps",
                                      name=f"scps_{rep}_{h}_{it}_{j}")
                        nc.tensor.matmul(sc, kT[:, j * 128:(j + 1) * 128],
                                         qT[:, it * 512:(it + 1) * 512],
                                         start=True, stop=True)
                        eu = eup.tile([128, 512], F16, tag="eu",
                                      name=f"eu_{rep}_{h}_{it}_{j}")
                        nc.scalar.activation(eu, sc, EXP, bias=bias_sh)
                        et = ep.tile([128, 512], F16, tag=f"e{u}",
                                     name=f"e_{rep}_{h}_{it}_{j}")
                        nc.vector.tensor_mul(et, eu, mts[(j, it)])
                        ets[(j, it)] = et
                    for it in its:
                        nc.tensor.matmul(aps[it], v_sb[j][:, h * 128:(h + 1) * 128],
                                         ets[(j, it)], start=(j == 0), stop=(j == JC - 1))
                # denominator second pass: ones stationary constant across a
                # 16-long same-bank run per it (ldweights amortized 16x)
                for it in its:
                    for j in range(JC):
                        nc.tensor.matmul(sps[it], ones_mat, ets[(j, it)],
                                         start=(j == 0), stop=(j == JC - 1))
                for it in its:
                    rec = rcp.tile([128, 512], F32, tag="rec",
                                   name=f"rec_{rep}_{h}_{it}")
                    nc.vector.reciprocal(rec, sps[it])
                    nc.vector.tensor_mul(attnT[h][:, it * 512:(it + 1) * 512],
                                         aps[it], rec)


def _phase3(nc, tc, attnT, woP, yT, wp, op, rep=0):
    """Row-parallel o_proj, wo stationary: one weight load per 4 matmuls.
    yT[o, i] = sum_dh wo[dh, o] * attnT[dh, i], stored f16 transposed."""
    OSUB = H // 128         # 40 output row tiles
    with tc.tile_pool(name=f"p3ps_{rep}", bufs=8, space="PSUM") as pp:
        for o in range(OSUB):
            wo = wp.tile([128, HPC, 128], F16, tag="wo", name=f"wo_{rep}_{o}")
            nc.gpsimd.dma_start(
                out=wo,
                in_=woP[o].rearrange("p (c m) -> p c m", m=128))
            ps = [pp.tile([128, 512], F32, tag="yps", name=f"yps_{rep}_{o}_{i}")
                  for i in range(4)]
            yo = op.tile([128, 4, 512], F16, tag="yo", name=f"yo_{rep}_{o}")
            # c-outer: one wo stationary feeds 4 consecutive matmuls (i banks)
            for c in range(HPC):
                for i in range(4):
                    nc.tensor.matmul(ps[i], wo[:, c, :],
                                     attnT[c][:, i * 512:(i + 1) * 512],
                                     start=(c == 0), stop=(c == HPC - 1))
            for i in range(4):
                if i % 2 == 0:
                    nc.scalar.copy(yo[:, i, :], ps[i])
                else:
                    nc.vector.tensor_copy(yo[:, i, :], ps[i])
            nc.gpsimd.dma_start(
                out=yT[o * 128:(o + 1) * 128, :],
                in_=yo.rearrange("p i t -> p (i t)"))


def build(repeat=1, phases=(1, 2, 3)):
    nc = bacc.Bacc("TRN2", target_bir_lowering=False, debug=False, num_devices=NCORES)
    # per-tile contiguous packed layouts (host pre-shuffles): every DMA reads
    # >=1.25KB contiguous per partition
    xP = nc.dram_tensor("xP", [NPAIR, 128, KC * PB], F16, kind="ExternalInput").ap()
    wP = nc.dram_tensor("wP", [3 * HPC, 128, KC * 128], F16, kind="ExternalInput").ap()
    woP = nc.dram_tensor("woP", [H // 128, 128, HPC * 128], F16, kind="ExternalInput").ap()
    emT = nc.dram_tensor("emT", [HPC, T, T], F16, kind="ExternalInput").ap()
    yT = nc.dram_tensor("yT", [H, T], F16, kind="ExternalOutput").ap()

    with tile.TileContext(nc) as tc:
        with tc.tile_pool(name="qkvp", bufs=1) as qp, \
             tc.tile_pool(name="vsbp", bufs=1) as vp, \
             tc.tile_pool(name="attnTp", bufs=1) as ap, \
             tc.tile_pool(name="constp", bufs=1) as cp:
            ones_f = cp.tile([128, 128], F32, name="ones_f")
            nc.vector.memset(ones_f, 1.0)
            ones_mat = cp.tile([128, 128], F16, name="ones_mat")
            nc.vector.tensor_copy(ones_mat, ones_f)
            ident_f = cp.tile([128, 128], F32, name="ident_f")
            make_identity(nc, ident_f)
            ident = cp.tile([128, 128], F16, name="ident")
            nc.vector.tensor_copy(ident, ident_f)
            bias_sh = cp.tile([128, 1], F32, name="bias_sh")
            nc.vector.memset(bias_sh, EXP_SHIFT_S)
            qkv_sb = [qp.tile([128, T], F16, name=f"qkv_{m}") for m in range(3 * HPC)]
            v_sb = [vp.tile([128, DPC], F16, name=f"vsb_{j}") for j in range(JC)]
            attnT = [ap.tile([128, T], F16, name=f"attnT_{c}") for c in range(HPC)]
            for rep in range(repeat):
                if 1 in phases:
                    _phase1(nc, tc, xP, wP, qkv_sb, v_sb, ident, rep)
                with tc.tile_pool(name=f"p3w_{rep}", bufs=2) as wp3, \
                     tc.tile_pool(name=f"p3o_{rep}", bufs=2) as op3:
                    if 2 in phases:
                        _phase2(nc, tc, qkv_sb, v_sb, emT, attnT, ones_mat,
                                bias_sh, rep)
                    if 3 in phases:
                        _phase3(nc, tc, attnT, woP, yT, wp3, op3, rep)
    nc.compile()
    return nc


_nc = None


def _get_nc():
    global _nc
    if _nc is None:
        _nc = build()
    return _nc


def make_in_maps(hidden_states, attention_mask, W_pack, o_proj_w):
    hs = np.ascontiguousarray(np.asarray(hidden_states, dtype=np.float32).reshape(T, H))
    mask = np.asarray(attention_mask, dtype=np.float32)
    wp = np.asarray(W_pack, dtype=np.float32)
    wo = np.asarray(o_proj_w, dtype=np.float32)

    xT = hs.T.astype(np.float16)                          # [H, T]
    # packed x: xP[n, p, k*PB+t] = xT[k*128+p, n*PB+t]
    xP = np.ascontiguousarray(
        xT.reshape(KC, 128, NPAIR, PB).transpose(2, 1, 0, 3)
        .reshape(NPAIR, 128, KC * PB))
    scale = np.float32(1.0 / math.sqrt(HD))
    wq = wp[0:H].reshape(NH, HD, H)
    wk = wp[H:2 * H].reshape(NH, HD, H)
    wv = wp[2 * H:3 * H].reshape(NH, HD, H)

    in_maps = []
    for c in range(NCORES):
        h0, h1 = c * HPC, (c + 1) * HPC
        # m-tile i holds (q|k|v of head i//3) matching M_ORDER's qkv_sb index
        blocks = []
        for h in range(h0, h1):
            blocks.append(wq[h] * scale)
            blocks.append(wk[h])
            blocks.append(wv[h])
        w_c = np.concatenate(blocks, axis=0)              # [1920, H]
        wqkvT_c = w_c.T.astype(np.float16)                # [H, 1920]
        # packed w: wP[m, p, k*128+col] = wqkvT[k*128+p, m*128+col]
        wP_c = np.ascontiguousarray(
            wqkvT_c.reshape(KC, 128, 3 * HPC, 128).transpose(2, 1, 0, 3)
            .reshape(3 * HPC, 128, KC * 128))
        woT_c = wo[:, h0 * HD:h1 * HD].T.astype(np.float16)   # [640, H]
        # packed wo: woP[o, p, cblk*128+col] = woT[cblk*128+p, o*128+col]
        woP_c = np.ascontiguousarray(
            woT_c.reshape(HPC, 128, H // 128, 128).transpose(2, 1, 0, 3)
            .reshape(H // 128, 128, HPC * 128))
        emT_c = np.ascontiguousarray(
            np.exp(mask[h0:h1].transpose(0, 2, 1) - 8.0).astype(np.float16))
        in_maps.append({"xP": xP, "wP": wP_c, "woP": woP_c, "emT": emT_c})
    return in_maps


_runner = None


def _cached_runner(nc):
    """Jit the bass_exec shard_map once so repeat kernel() calls skip the
    walrus/NEFF recompile that a fresh run_bass_kernel_spmd would pay."""
    import jax
    from jax.experimental.shard_map import shard_map
    from jax.sharding import Mesh, PartitionSpec
    from concourse import bass2jax

    bass2jax.install_neuronx_cc_hook()
    partition_name = nc.partition_id_tensor.name if nc.partition_id_tensor else None
    in_names, out_names, out_avals, zero_outs = [], [], [], []
    for alloc in nc.m.functions[0].allocations:
        if not isinstance(alloc, mybir.MemoryLocationSet):
            continue
        name = alloc.memorylocations[0].name
        if alloc.kind == "ExternalInput":
            if name != partition_name:
                in_names.append(name)
        elif alloc.kind == "ExternalOutput":
            out_names.append(name)
            shape = tuple(alloc.tensor_shape)
            dtype = mybir.dt.np(alloc.dtype)
            out_avals.append(jax.core.ShapedArray(shape, dtype))
            zero_outs.append(np.zeros(shape, dtype))
    all_in = list(in_names) + list(out_names)
    if partition_name is not None:
        all_in.append(partition_name)

    def _body(*args):
        operands = list(args)
        if partition_name is not None:
            operands.append(bass2jax.partition_id_tensor())
        outs = bass2jax._bass_exec_p.bind(
            *operands, out_avals=tuple(out_avals), in_names=tuple(all_in),
            out_names=tuple(out_names), lowering_input_output_aliases=(),
            sim_require_finite=True, sim_require_nnan=True, nc=nc)
        return tuple(outs)

    mesh = Mesh(np.asarray(jax.devices()[:NCORES]), ("core",))
    n_args = len(in_names) + len(out_names)
    fn = jax.jit(shard_map(_body, mesh=mesh,
                           in_specs=(PartitionSpec("core"),) * n_args,
                           out_specs=(PartitionSpec("core"),) * len(out_names),
                           check_rep=False), keep_unused=True)

    def run(in_maps):
        args = [np.concatenate([np.asarray(m[n]) for m in in_maps], axis=0)
                for n in in_names]
        args += [np.zeros((NCORES * z.shape[0], *z.shape[1:]), z.dtype)
                 for z in zero_outs]
        outs = fn(*args)
        return [{name: np.asarray(outs[i]).reshape(NCORES, *out_avals[i].shape)[c]
                 for i, name in enumerate(out_names)} for c in range(NCORES)]

    return run


def kernel(input_pos=None, end=None, hidden_states=None, attention_mask=None,
           W_pack=None, o_proj_w=None, k_cache=None, v_cache=None):
    # input_pos == arange(T) and end == T per the problem spec, so the KV
    # cache write is a full overwrite and the zero-filled caches never
    # contribute to the output — both are intentionally unused here.
    global _runner
    in_maps = make_in_maps(hidden_states, attention_mask, W_pack, o_proj_w)
    nc = _get_nc()
    if _runner is None:
        results = run_bass_kernel_spmd(nc, in_maps, list(range(NCORES))).results
        _runner = _cached_runner(nc)
    else:
        results = _runner(in_maps)
    y = results[0]["yT"].astype(np.float32)
    for c in range(1, NCORES):
        y = y + results[c]["yT"]
    return np.ascontiguousarray(y.T).reshape(1, T, H)

